# revision 27
# baseline (speedup 1.0000x reference)
"""Trainium2 Bass kernel for agent attention (sparse_attention problem).

Per-core work (data-parallel over batch B=8 across 8 NeuronCores):
  x[b] [256, 64, 64] -> qkv 3x3 conv (dif-conv + BN folded into weights)
  -> agent attention (8 heads, d=32, 64 agent tokens)
  -> depthwise 3x3 pe conv on v -> 1x1 proj.

Conv runs in float32r (1 col/cycle); attention tensors are bf16.
Stage-1 (agents attend to keys) and stage-2 score/exp work are emitted
inside the conv groups' instruction windows so the PE array never idles
long enough for the HAM clock gate to drop it to 1.2 GHz.  The stage-2
tail normalizes exp-scores by the agent-sum (PE broadcast matmul), then
aggregates straight into the [channel, pixel] accumulator orientation so
no output transposes are needed.
"""
import numpy as np
import ml_dtypes

NUM_HEADS = 8
AGENT_NUM = 64
THETA = 0.7
C = 256
H = W = 64
HW = H * W
D = C // NUM_HEADS          # 32
PS = 8                      # pool size
N_CORES = 8
B = 8

_cache = {}
DBG = False

# const blob column layout (f32 words)
_BQ0, _PEW0, _PB0, _IDN0, _PW0, _SEL0, _CBLOB = 0, 6, 24, 26, 90, 346, 410


def _build():
    import concourse.bass as bass
    import concourse.tile as tile
    from concourse import bacc, mybir
    from contextlib import ExitStack

    f32 = mybir.dt.float32
    f32r = mybir.dt.float32r
    bf16 = mybir.dt.bfloat16
    AF = mybir.ActivationFunctionType
    ALU = mybir.AluOpType
    AX = mybir.AxisListType

    nc = bacc.Bacc("TRN2", target_bir_lowering=False, debug=False,
                   enable_asserts=True, num_devices=N_CORES)

    X = nc.dram_tensor("x", [2, 128, H, W], f32r, kind="ExternalInput").ap()
    WQ = nc.dram_tensor("wq", [6, 128, 2, 9, 128], f32r,
                        kind="ExternalInput").ap()
    CB = nc.dram_tensor("cb", [128, _CBLOB], f32, kind="ExternalInput").ap()
    IDN = nc.dram_tensor("idn", [128, 128], bf16, kind="ExternalInput").ap()
    PW = nc.dram_tensor("pw", [128, 2, 256], bf16, kind="ExternalInput").ap()
    SEL = nc.dram_tensor("sel", [2, 128], bf16, kind="ExternalInput").ap()
    OUT = nc.dram_tensor("out", [2, 128, HW], f32, kind="ExternalOutput").ap()
    if DBG:
        DAT1 = nc.dram_tensor("dat1", [66, 512], bf16,
                              kind="ExternalOutput").ap()
        DAZT = nc.dram_tensor("dazt", [128, 256], bf16,
                              kind="ExternalOutput").ap()
        DATT = nc.dram_tensor("datt", [128, 66 * 66], bf16,
                              kind="ExternalOutput").ap()

    # softmax exp scale: d^-0.5, with the 1/64 agent-pool mean folded in
    SCALE = (D ** -0.5) / (PS * PS)
    PADF = 66 * 66  # 4356

    with tile.TileContext(nc) as tc:
        with ExitStack() as top:
            pers = top.enter_context(tc.tile_pool(name="pers", bufs=1))
            x_pad = [pers.tile([128, PADF], f32r, tag=f"xp{i}", name=f"xp{i}")
                     for i in range(2)]
            q_sb = [pers.tile([128, HW], bf16, tag=f"q{i}", name=f"q{i}")
                    for i in range(2)]
            k_sb = [pers.tile([128, HW], bf16, tag=f"k{i}", name=f"k{i}")
                    for i in range(2)]
            v_pad = [pers.tile([128, PADF], bf16, tag=f"vp{i}", name=f"vp{i}")
                     for i in range(2)]
            att = [pers.tile([128, PADF], bf16, tag=f"ao{i}", name=f"ao{i}")
                   for i in range(2)]
            cb = pers.tile([128, _CBLOB], f32, tag="cb", name="cb")
            asum = pers.tile([128, 128], f32, tag="asum", name="asum")
            abd = pers.tile([128, 512], bf16, tag="abd", name="abd")
            azt = pers.tile([128, 256], bf16, tag="azt", name="azt")
            at1s = pers.tile([66, 512], bf16, tag="at1s", name="at1s")
            ones2 = pers.tile([128, 2], bf16, tag="on2", name="on2")
            idn_t = pers.tile([128, 128], bf16, tag="idn", name="idn")
            pw_t = pers.tile([128, 2, 256], bf16, tag="pw", name="pw")
            sel_t = pers.tile([2, 128], bf16, tag="sel", name="sel")
            e2pool = top.enter_context(tc.tile_pool(name="e2p", bufs=32))

            nc.sync.dma_start(cb[:], CB[:])
            nc.sync.dma_start(idn_t[:], IDN[:])
            nc.sync.dma_start(pw_t[:], PW[:])
            nc.sync.dma_start(sel_t[:], SEL[:])
            bq = cb[:, _BQ0:_BQ0 + 6]
            pew = cb[:, _PEW0:_PEW0 + 18].rearrange(
                "p (a b) -> p a b", a=2, b=9)
            pb = cb[:, _PB0:_PB0 + 2]
            idn = idn_t[:]
            pwv = pw_t[:]
            sel2 = sel_t[:]

            nc.vector.memset(ones2[:], 0.0)
            nc.vector.memset(ones2[0:64, 0:1], 1.0)
            nc.vector.memset(ones2[64:128, 1:2], 1.0)
            nc.vector.memset(azt[:], 0.0)

            # padded-x borders + chunked input DMA (16-row blocks)
            for cc in range(2):
                xv = x_pad[cc][:].bitcast(f32).rearrange(
                    "p (r c) -> p r c", r=66, c=66)
                nc.vector.memset(xv[:, 0:1, :], 0.0)
                nc.vector.memset(xv[:, 65:66, :], 0.0)
                nc.vector.memset(xv[:, :, 0:1], 0.0)
                nc.vector.memset(xv[:, :, 65:66], 0.0)
            for ck in range(4):
                for cc in range(2):
                    xvr = x_pad[cc][:].rearrange(
                        "p (r c) -> p r c", r=66, c=66)
                    nc.sync.dma_start(
                        xvr[:, 1 + 16 * ck:17 + 16 * ck, 1:65],
                        X[cc, :, 16 * ck:16 * ck + 16, :])
            for cc in range(2):
                vv = v_pad[cc][:].rearrange("p (r c) -> p r c", r=66, c=66)
                nc.vector.memset(vv[:, 0:1, :], 0.0)
                nc.vector.memset(vv[:, 65:66, :], 0.0)
                nc.vector.memset(vv[:, :, 0:1], 0.0)
                nc.vector.memset(vv[:, :, 65:66], 0.0)

            units = [(nt, cc, half) for nt in range(8)
                     for cc in range(2) for half in range(2)]
            e2s_map = {}

            # ---- conv + stage 1 + stage-2 score/exp ----
            with ExitStack() as ph:
                cps = ph.enter_context(
                    tc.tile_pool(name="cps", bufs=2, space="PSUM"))
                wpool = ph.enter_context(tc.tile_pool(name="wq", bufs=1))
                trps = ph.enter_context(
                    tc.tile_pool(name="trps", bufs=1, space="PSUM"))
                sp1ps = ph.enter_context(
                    tc.tile_pool(name="sp1", bufs=2, space="PSUM"))
                at1ps = ph.enter_context(
                    tc.tile_pool(name="at1", bufs=1, space="PSUM"))
                sp2ps = ph.enter_context(
                    tc.tile_pool(name="sp2", bufs=2, space="PSUM"))
                s1sb = ph.enter_context(tc.tile_pool(name="s1sb", bufs=2))

                def conv_group(mc, post_rb=None):
                    wts = []
                    for kc in range(2):
                        wt = wpool.tile([128, 9, 128], f32r, tag="w",
                                        name="w", bufs=4)
                        nc.sync.dma_start(wt[:], WQ[mc, :, kc])
                        wts.append(wt)
                    for rb in range(8):
                        ps_t = cps.tile([128, 512], f32, tag="cps",
                                        name="cpst")
                        psv = ps_t[:].rearrange("p (r c) -> p r c", r=8, c=64)
                        i = 0
                        for kc in range(2):
                            xv = x_pad[kc][:].rearrange(
                                "p (r c) -> p r c", r=66, c=66)
                            for s in range(9):
                                ky, kx = s // 3, s % 3
                                rhs = xv[:, 8 * rb + ky: 8 * rb + ky + 8,
                                         kx: kx + 64]
                                nc.tensor.matmul(
                                    psv, wts[kc][:, s, :], rhs,
                                    start=(i == 0), stop=(i == 17))
                                i += 1
                        bias = bq[:, mc: mc + 1]
                        if mc < 2:
                            dst = q_sb[mc][:, 512 * rb: 512 * (rb + 1)]
                            nc.vector.tensor_scalar_add(dst, ps_t[:], bias)
                        elif mc < 4:
                            dst = k_sb[mc - 2][:, 512 * rb: 512 * (rb + 1)]
                            nc.vector.tensor_scalar_add(dst, ps_t[:], bias)
                        else:
                            vv = v_pad[mc - 4][:].rearrange(
                                "p (r c) -> p r c", r=66, c=66)
                            dst = vv[:, 8 * rb + 1: 8 * rb + 9, 1:65]
                            nc.vector.tensor_scalar_add(dst, psv, bias)
                        if post_rb is not None:
                            post_rb(rb)

                # stage-2 fronts: scores + exp (only need q and abd)
                def front(u):
                    nt, cc, half = u
                    sp = sp2ps.tile([128, 512], f32, tag="sp2", name="sp2t")
                    nc.tensor.matmul(
                        sp[:],
                        abd[:, 256 * cc + 128 * half:
                            256 * cc + 128 * (half + 1)],
                        q_sb[cc][:, 512 * nt:512 * (nt + 1)],
                        start=True, stop=True)
                    e2 = e2pool.tile([128, 512], bf16, tag="e2", name="e2",
                                     bufs=32)
                    nc.scalar.activation(e2[:], sp[:], AF.Exp, scale=SCALE)
                    e2s_map[u] = e2

                # depthwise pe conv on v (flat shifted adds on DVE),
                # writing the padded attention-output accumulator
                def pe_ops(cc):
                    ops = []
                    for s in range(9):
                        off = 66 * (s // 3 - 1) + (s % 3 - 1)
                        src = v_pad[cc][:, 67 + off:4289 + off]
                        dst = att[cc][:, 67:4289]
                        w_s = pew[:, cc, s:s + 1]
                        if s == 0:
                            ops.append(lambda d=dst, sv=src, w=w_s:
                                       nc.vector.tensor_scalar_mul(d, sv, w))
                        else:
                            ops.append(lambda d=dst, sv=src, w=w_s:
                                       nc.vector.scalar_tensor_tensor(
                                           d, sv, w, d, ALU.mult, ALU.add))
                    return ops

                pe0 = pe_ops(0)
                pe1 = pe_ops(1)

                # q
                conv_group(0)
                conv_group(1)

                # pooled agent tokens (2-stage reduce) + block-diag a
                for cc in range(2):
                    r1t = s1sb.tile([128, 512], f32, tag="pr", name="prt")
                    qv = q_sb[cc][:].rearrange(
                        "p (by dy bx dx) -> p by dy bx dx",
                        by=8, dy=8, bx=8, dx=8)
                    nc.vector.tensor_reduce(r1t[:], qv, AX.X, ALU.add)
                    rv = r1t[:].rearrange("p (by dy bx) -> p by bx dy",
                                          by=8, dy=8, bx=8)
                    nc.vector.tensor_reduce(
                        asum[:, 64 * cc:64 * (cc + 1)], rv, AX.X, ALU.add)
                nc.vector.memset(abd[:], 0.0)
                for cc in range(2):
                    for j in range(4):
                        nc.vector.tensor_copy(
                            abd[32 * j:32 * (j + 1),
                                256 * cc + 64 * j:256 * cc + 64 * (j + 1)],
                            asum[32 * j:32 * (j + 1),
                                 64 * cc:64 * cc + 64])

                # v (pe conv cc0 + stage-2 fronts drip into the windows)
                conv_group(4, post_rb=lambda rb: front(units[rb]))

                def post_v1(rb):
                    pe0[rb]()
                    front(units[8 + rb])
                conv_group(5, post_rb=post_v1)
                pe0[8]()

                # transposed v chunks [pix, ch] (+ones cols for Z1)
                vts = [None] * 32

                def make_vt4(rb):
                    trt = trps.tile([128, 512], f32, tag="tr", name="trt")
                    trtb = trt[:].bitcast(bf16)
                    for jj in range(4):
                        ch = 4 * rb + jj
                        vtc = s1sb.tile([128, 264], bf16, tag="vt",
                                        name="vt", bufs=32)
                        vts[ch] = vtc
                        vtv = vtc[:].rearrange("p (a b) -> p a b", a=4, b=66)
                        nc.gpsimd.memset(vtv[:, :, 64:66], 1.0)
                        for cc in range(2):
                            vv = v_pad[cc][:].rearrange(
                                "p (r c) -> p r c", r=66, c=66)
                            vstg = s1sb.tile([128, 128], bf16, tag="vstg",
                                             name="vstg", bufs=6)
                            nc.gpsimd.tensor_copy(
                                vstg[:].rearrange("p (r c) -> p r c",
                                                  r=2, c=64),
                                vv[:, 2 * ch + 1: 2 * ch + 3, 1:65])
                            nc.tensor.matmul(
                                trtb[:, 256 * jj + 128 * cc:
                                     256 * jj + 128 * (cc + 1)],
                                vstg[:], idn, is_transpose=True,
                                skip_group_check=True)
                        nc.scalar.copy(
                            vtv[:, 0:4, 0:64],
                            trtb[:, 256 * jj:256 * (jj + 1)].rearrange(
                                "p (a b) -> p a b", a=4, b=64))

                # stage 1: scores (k vs agents) + aggregation
                at1 = at1ps.tile([66, 512], f32, tag="at1", name="at1t")
                ets = {}

                def stage1_sp4(ch0, cc):
                    for ch in range(ch0, ch0 + 4):
                        sp = sp1ps.tile([128, 256], f32, tag="sp",
                                        name="spt")
                        nc.tensor.matmul(
                            sp[:], k_sb[cc][:, 128 * ch:128 * (ch + 1)],
                            abd[:, 256 * cc:256 * (cc + 1)],
                            start=True, stop=True)
                        et = s1sb.tile([128, 256], bf16, tag="et",
                                       name="et", bufs=8)
                        nc.scalar.activation(et[:], sp[:], AF.Exp,
                                             scale=SCALE)
                        ets[(ch, cc)] = et

                # NOTE: start=True clears has_written for the whole PSUM
                # bank, so only the very first agg matmul may carry it.
                def stage1_agg4(ch0, cc):
                    for ch in range(ch0, ch0 + 4):
                        et = ets.pop((ch, cc))
                        for half in range(2):
                            hp = 2 * cc + half
                            nc.tensor.matmul(
                                at1[:, 128 * hp:128 * (hp + 1)],
                                vts[ch][:, 66 * hp:66 * hp + 66],
                                et[:, 128 * half:128 * (half + 1)],
                                start=(ch == 0 and hp == 0),
                                stop=(ch == 31),
                                skip_group_check=True)

                def post_k0(rb):
                    if rb > 0:
                        stage1_sp4(4 * (rb - 1), 0)
                    make_vt4(rb)
                    if rb > 0:
                        stage1_agg4(4 * (rb - 1), 0)
                    front(units[16 + rb])
                    pe1[rb]()

                def post_k1(rb):
                    if rb == 0:
                        stage1_sp4(28, 0)
                    stage1_sp4(4 * rb, 1)
                    if rb == 0:
                        stage1_agg4(28, 0)
                        pe1[8]()
                    else:
                        stage1_agg4(4 * (rb - 1), 1)
                    front(units[24 + rb])

                conv_group(2, post_rb=post_k0)
                conv_group(3, post_rb=post_k1)
                stage1_agg4(28, 1)

                # stage-1 result psum -> sbuf
                nc.scalar.copy(at1s[:], at1[:])
                if DBG:
                    nc.sync.dma_start(DAT1[:], at1s[:])

            # ---- stage-2 tail: normalize + aggregate + proj ----
            with ExitStack() as ph:
                s2sb = ph.enter_context(tc.tile_pool(name="s2sb", bufs=2))

                # finalize stage-1: transpose + normalize by Z1
                with tc.tile_pool(name="tpp", bufs=1, space="PSUM") as tpp:
                    tpt = tpp.tile([128, 132], f32, tag="tp", name="tpt")
                    tpb = tpt[:].bitcast(bf16)
                    for hp in range(4):
                        nc.tensor.matmul(
                            tpb[:, 66 * hp:66 * hp + 66],
                            at1s[:, 128 * hp:128 * (hp + 1)],
                            idn[0:66, 0:66], is_transpose=True,
                            skip_group_check=True)
                    for hp in range(4):
                        r1 = s2sb.tile([128, 1], f32, tag="r1", name="r1")
                        nc.vector.reciprocal(
                            r1[:], tpb[:, 66 * hp + 64:66 * hp + 65])
                        nc.vector.tensor_scalar_mul(
                            azt[0:64, 64 * hp:64 * hp + 32],
                            tpb[0:64, 66 * hp:66 * hp + 32], r1[0:64, :])
                        nc.vector.tensor_scalar_mul(
                            azt[64:128, 64 * hp + 32:64 * hp + 64],
                            tpb[64:128, 66 * hp + 32:66 * hp + 64],
                            r1[64:128, :])

                with ExitStack() as ph2:
                    zps = ph2.enter_context(
                        tc.tile_pool(name="zps", bufs=2, space="PSUM"))
                    bcps = ph2.enter_context(
                        tc.tile_pool(name="bcps", bufs=2, space="PSUM"))
                    gpps = ph2.enter_context(
                        tc.tile_pool(name="gpps", bufs=2, space="PSUM"))
                    prps = ph2.enter_context(
                        tc.tile_pool(name="prps", bufs=1, space="PSUM"))

                    cur = {}

                    def m_z(u):
                        z = zps.tile([2, 512], f32, tag="z", name="zt")
                        nc.tensor.matmul(z[:], ones2[:], e2s_map[u][:],
                                         start=True, stop=True)
                        rz = s2sb.tile([2, 512], f32, tag="rz", name="rz",
                                       bufs=3)
                        nc.vector.reciprocal_approx_fast(rz[:], z[:])
                        rzb = s2sb.tile([2, 512], bf16, tag="rzb",
                                        name="rzb", bufs=3)
                        nc.scalar.copy(rzb[:], rz[:])
                        cur[("rzb", u)] = rzb

                    def m_bc(u):
                        rzb = cur.pop(("rzb", u))
                        bc2 = bcps.tile([128, 512], f32, tag="bc",
                                        name="bct")
                        nc.tensor.matmul(bc2[:], sel2, rzb[:],
                                         start=True, stop=True)
                        e2 = e2s_map.pop(u)
                        es = s2sb.tile([128, 512], bf16, tag="es",
                                       name="es", bufs=3)
                        nc.vector.tensor_tensor(es[:], bc2[:], e2[:],
                                                ALU.mult)
                        cur[("es", u)] = es

                    def m_gp(u):
                        nt, cc, half = u
                        hp = 2 * cc + half
                        if half == 0:
                            cur[("gp", nt, cc)] = gpps.tile(
                                [128, 512], f32, tag="gp", name="gpt")
                        gp_t = cur[("gp", nt, cc)]
                        es = cur.pop(("es", u))
                        nc.tensor.matmul(
                            gp_t[64 * half:64 * (half + 1), :],
                            azt[:, 64 * hp:64 * (hp + 1)], es[:],
                            start=True, stop=True, skip_group_check=True)

                    def back(u):
                        nt, cc, half = u
                        if half != 1:
                            return
                        gp_t = cur.pop(("gp", nt, cc))
                        av = att[cc][:].rearrange(
                            "p (r c) -> p r c", r=66, c=66)[
                            :, 1 + 8 * nt:9 + 8 * nt, 1:65]
                        gv = gp_t[:].rearrange("p (r c) -> p r c",
                                               r=8, c=64)
                        nc.vector.tensor_tensor(av, gv, av, ALU.add)
                        if cc == 1:
                            for mc in range(2):
                                pp = prps.tile([128, 512], f32, tag="pp",
                                               name="ppt")
                                for kc in range(2):
                                    rv = att[kc][:].rearrange(
                                        "p (r c) -> p r c", r=66, c=66)[
                                        :, 1 + 8 * nt:9 + 8 * nt, 1:65]
                                    nc.tensor.matmul(
                                        pp[:],
                                        pwv[:, kc, 128 * mc:128 * (mc + 1)],
                                        rv, start=(kc == 0), stop=(kc == 1))
                                ot = s2sb.tile([128, 512], f32, tag="ot",
                                               name="ott", bufs=2)
                                nc.vector.tensor_scalar_add(
                                    ot[:], pp[:], pb[:, mc:mc + 1])
                                nc.sync.dma_start(
                                    OUT[mc, :, 512 * nt:512 * (nt + 1)],
                                    ot[:])

                    nu = len(units)
                    for i in range(0, nu + 4):
                        if i < nu:
                            m_z(units[i])
                        if 0 <= i - 2 < nu:
                            m_bc(units[i - 2])
                        if 0 <= i - 3 < nu:
                            m_gp(units[i - 3])
                        if 0 <= i - 4 < nu:
                            back(units[i - 4])
                    if DBG:
                        nc.sync.dma_start(DAZT[:], azt[:])
                        nc.sync.dma_start(DATT[:], att[0][:])

    nc.compile()
    return nc


def _prep_consts(qkv_w, qkv_s, qkv_b, pe_w, pe_s, pe_b, proj_w, proj_s,
                 proj_b):
    f = np.float32
    bf = ml_dtypes.bfloat16
    w = np.asarray(qkv_w, f).copy()          # [768, 256, 3, 3]
    dif = (w[:, :, 0, 1] + w[:, :, 1, 0] + w[:, :, 1, 1] + w[:, :, 1, 2]
           + w[:, :, 2, 1])
    w[:, :, 1, 1] -= THETA * dif
    w *= np.asarray(qkv_s, f)[:, None, None, None]
    # WQ[mc, p, kc, s, o'] = w[128*mc+o', 128*kc+p, s//3, s%3]
    wq = w.reshape(6, 128, 2, 128, 9)        # [mc, o', kc, p, s]
    wq = np.ascontiguousarray(wq.transpose(0, 3, 2, 4, 1))  # [6,128,2,9,128]

    bq = np.ascontiguousarray(np.asarray(qkv_b, f).reshape(6, 128).T)

    pe_wf = np.asarray(pe_w, f)[:, 0] * np.asarray(pe_s, f)[:, None, None]
    pew = np.zeros((128, 2, 9), f)
    for kc in range(2):
        for s in range(9):
            pew[:, kc, s] = pe_wf[128 * kc:128 * (kc + 1), s // 3, s % 3]

    pwm = np.asarray(proj_w, f)[:, :, 0, 0] * np.asarray(proj_s, f)[:, None]
    # pw[p, kc, o] = pwm[o, 128*kc + p]
    pw = np.ascontiguousarray(
        pwm.T.reshape(2, 128, 256).transpose(1, 0, 2)).astype(bf)
    pbv = np.asarray(proj_b, f) + pwm @ np.asarray(pe_b, f)
    pb = np.ascontiguousarray(pbv.reshape(2, 128).T)

    idn = np.eye(128, dtype=bf)
    sel2 = np.zeros((128, 128), bf)
    sel2[0, 0:64] = 1.0
    sel2[1, 64:128] = 1.0

    def pack_bf(x16):
        # pack [128, 2n] bf16 -> [128, n] uint32 (little-endian pairs)
        u = np.ascontiguousarray(x16).view(np.uint16).astype(np.uint32)
        return u[:, 0::2] | (u[:, 1::2] << 16)

    blob = np.zeros((128, _CBLOB), np.uint32)
    blob[:, _BQ0:_BQ0 + 6] = bq.view(np.uint32)
    blob[:, _PEW0:_PEW0 + 18] = pew.reshape(128, 18).view(np.uint32)
    blob[:, _PB0:_PB0 + 2] = pb.view(np.uint32)
    return dict(wq=wq, cb=blob.view(np.float32), idn=idn, pw=pw,
                sel=np.ascontiguousarray(sel2[0:2, :]))


def kernel(x, qkv_w, qkv_s, qkv_b, pe_w, pe_s, pe_b, proj_w, proj_s, proj_b):
    from concourse.bass_utils import run_bass_kernel_spmd

    if "nc" not in _cache:
        _cache["nc"] = _build()
    nc = _cache["nc"]

    consts = _prep_consts(qkv_w, qkv_s, qkv_b, pe_w, pe_s, pe_b, proj_w,
                          proj_s, proj_b)
    x = np.asarray(x, np.float32)
    in_maps = []
    for b in range(B):
        m = dict(consts)
        m["x"] = np.ascontiguousarray(x[b].reshape(2, 128, H, W))
        in_maps.append(m)

    res = run_bass_kernel_spmd(nc, in_maps, list(range(N_CORES)), trace=False)
    out = np.empty((B, C, H, W), np.float32)
    for b in range(B):
        out[b] = res.results[b]["out"].reshape(C, H, W)
    return out
